# revision 18
# baseline (speedup 1.0000x reference)
"""Causal self-attention (B=4, T=2048, C=1024, H=16) on 8 Trainium2 NeuronCores.

Core index = 2*batch + head_group: each core owns one batch element and 8 of
the 16 heads (tensor-parallel split of c_attn output dim / c_proj input dim).
Each core emits a partial projection out^T [C, T]; the host sums the two
head-group partials per batch and adds the bias terms.

v2 (vs v1 baseline 358us):
  * Head-PAIR processing: even local head lives on SBUF partitions 0-63,
    odd head on 64-127. The two K=64 scores matmuls of a pair are emitted
    back-to-back with inferred tile_position (0,0)/(64,0) -> they run
    CONCURRENTLY in the PE array (row-group tiling), and their LDWEIGHTS
    cross-hide against the other row group's matmul.
  * No negI mask matmuls: causal masking is a post-exp DVE multiply of the
    128-wide diagonal block only (PE -21us, DVE +16us).
  * qkv projection (phase B), attention (phase C) and out-projection are
    interleaved in emission order so the Tile scheduler can fill PE stalls
    (C is ACT-exp-bound in bursts) with independent B/proj matmuls.
  * x DMA-transposes split across the two HWDGE queues (sync+scalar) and
    issued first: PE starts ~6us in instead of ~24us.
  * normalize reads U'/rowsum directly from PSUM (no staging copies).

fp16 datapath (fp32 PSUM accumulation everywhere, fp32 softmax denominator).

Per-core pipeline per head pair u (heads 2u, 2u+1), per 512-wide i-chunk ic:
  S^T[j, i] = k_h^T q_h  for both heads -> one psum tile [128, 2, 512]
  P = exp(S^T / 8)       (one ACT op per (jt, ic) sub, both heads)
  diagonal j-tile: P *= keep (DVE, precomputed lower-tri mask)
  U'^T [65, i] (+)= [v_h|1]^T P_h^T  over j-tiles (ones col => rowsum row 64)
  yT[u-block, i] = U'[0:64] * bcast(1/rowsum)
out^T = W_p^T yT -> fp32 psum -> DVE copy -> DMA
"""

import numpy as np

import concourse.bass as bass
import concourse.mybir as mybir
import concourse.tile as tile
from concourse import bacc, bass_utils

B, T, C, H = 4, 2048, 1024, 16
HD = C // H          # 64 head dim
N_CORES = 8
HG = H // 2          # 8 heads per core
CL = HG * HD         # 512 local width of q/k/v
TT = T // 128        # 16 t-tiles
CB = C // 128        # 8 c-tiles
DB = CL // 128       # 4 local-hd tiles
NIC = T // 512       # i-chunks (4)
NP = HG // 2         # head pairs per core (4)

f32 = mybir.dt.float32
f16 = mybir.dt.float16

_PROG_CACHE = {}


def _emit(tc, aps):
    nc = tc.nc
    Exp = mybir.ActivationFunctionType.Exp
    Copy = mybir.ActivationFunctionType.Copy

    x_ap = aps["x"]
    wqk_ap = aps["wqk"]
    wv_ap = aps["wv"]
    wp_ap = aps["wp"]
    bqk_ap = aps["bqk"]
    keep2_ap = aps["keep2"]
    outT_ap = aps["outT"]

    from contextlib import ExitStack

    with ExitStack() as outer:
        const = outer.enter_context(tc.tile_pool(name="const", bufs=1))
        p_xT = outer.enter_context(tc.tile_pool(name="xT", bufs=1))
        p_qkT = outer.enter_context(tc.tile_pool(name="qkT", bufs=1))
        p_v = outer.enter_context(tc.tile_pool(name="vv", bufs=1))
        p_yT = outer.enter_context(tc.tile_pool(name="yT", bufs=1))
        p_w = outer.enter_context(tc.tile_pool(name="wsb", bufs=1))
        p_p = outer.enter_context(tc.tile_pool(name="pp", bufs=8))
        p_rb = outer.enter_context(tc.tile_pool(name="rb", bufs=3))
        p_ost = outer.enter_context(tc.tile_pool(name="ost", bufs=4))
        ps_b = outer.enter_context(tc.tile_pool(name="ps_b", bufs=2, space="PSUM"))
        ps_sc = outer.enter_context(tc.tile_pool(name="ps_sc", bufs=2, space="PSUM"))
        ps_u = outer.enter_context(tc.tile_pool(name="ps_u", bufs=2, space="PSUM"))

        # ---- DMAs: x transposes are the PE critical path -> they own both
        # HWDGE queues (sync+scalar), with t-chunk 0 split across the two so
        # its descriptor GENERATION (~6us each) runs in parallel. All weight
        # loads go through the gpsimd SWDGE queue (cheap generation).
        # x transposes: XBAR transfer is fast (~14ns/16x128 tile); the cost is
        # per-instruction DGE latency, which pipelines across the two HWDGE
        # queues. Small first pieces unblock B(0) early.
        # The XBAR serves transposes FIFO across queues: keep sync's queue to
        # ONE small piece so scalar's chunk-0 half isn't stuck behind a big
        # sync transfer; everything else streams on scalar in order.
        xT = p_xT.tile([128, CB, T], f16)
        nc.sync.dma_start_transpose(xT[:, :, 0:256], x_ap[0:256, :])
        nc.scalar.dma_start_transpose(xT[:, :, 256:512], x_ap[256:512, :])
        nc.scalar.dma_start_transpose(xT[:, :, 512:1024], x_ap[512:1024, :])
        nc.scalar.dma_start_transpose(xT[:, :, 1024:1536], x_ap[1024:1536, :])
        nc.scalar.dma_start_transpose(xT[:, :, 1536:2048], x_ap[1536:2048, :])
        # Weights ride the sync HWDGE (pipelines gen/transfer; regular DMAs
        # don't touch the XBAR) behind its single transpose, in first-use
        # order. Tiny constants on gpsimd SWDGE land ~10us.
        wqk_sb = p_w.tile([128, CB, CB * 128], f16)  # [c-part, cb, co*128+q]
        nc.sync.dma_start(
            wqk_sb[:, :, 0 : 4 * 128],
            wqk_ap[:, 0 : 4 * 128].rearrange("(cb p) n -> p cb n", p=128),
        )
        nc.sync.dma_start(
            wqk_sb[:, :, 4 * 128 : 8 * 128],
            wqk_ap[:, 4 * 128 : 8 * 128].rearrange("(cb p) n -> p cb n", p=128),
        )
        wv_sb = p_w.tile([128, CB, CL], f16)
        nc.sync.dma_start(wv_sb[:], wv_ap.rearrange("(cb p) n -> p cb n", p=128))
        wp_sb = p_w.tile([128, DB, C], f16)
        nc.sync.dma_start(wp_sb[:], wp_ap.rearrange("(db p) c -> p db c", p=128))
        keep2 = const.tile([128, 2, 128], f16)   # keep[j, ix, i] = (j <= i)
        nc.gpsimd.dma_start(keep2[:], keep2_ap)
        bqk = const.tile([128, CB], f32)
        nc.gpsimd.dma_start(bqk[:], bqk_ap.rearrange("co p -> p co"))

        # ---- PE warm-up: dep-free matmuls on a zeroed tile keep the PE busy
        # from t~5us so HAM un-throttles to 2.4 GHz before real work arrives,
        # and the array never sits cold waiting for the first x chunk.
        warm = const.tile([128, 512], f16)
        nc.vector.memset(warm[:], 0.0)
        wups = ps_u.tile([HD + 1, 512], f32, tag="u", name="wups")
        for _ in range(18):
            nc.tensor.matmul(
                wups[:], warm[:, 0 : HD + 1], warm[:], start=True, stop=True
            )

        # per-(co, tn) qkT tiles, per-jt v' tiles, per-tn yT tiles
        qkT = {}
        for co in range(CB):
            for tn in range(NIC):
                qkT[(co, tn)] = p_qkT.tile(
                    [128, 512], f16, tag=f"qkT_{co}_{tn}", name=f"qkT_{co}_{tn}"
                )
        vv = {}
        for jt in range(TT):
            vv[jt] = p_v.tile([128, HG, HD + 1], f16, tag=f"vv_{jt}", name=f"vv_{jt}")
            nc.vector.memset(vv[jt][:, :, HD : HD + 1], 1.0)
        yTn = {}
        for tn in range(NIC):
            yTn[tn] = p_yT.tile([128, DB, 512], f16, tag=f"yT_{tn}", name=f"yT_{tn}")

        def emit_qkT_group(co, tn):
            ps = ps_b.tile([128, 512], f32, tag="bps")
            for cb in range(CB):
                nc.tensor.matmul(
                    ps[:],
                    wqk_sb[:, cb, co * 128 : (co + 1) * 128],
                    xT[:, cb, tn * 512 : (tn + 1) * 512],
                    start=(cb == 0),
                    stop=(cb == CB - 1),
                )
            nc.vector.tensor_scalar_add(qkT[(co, tn)][:], ps[:], bqk[:, co : co + 1])

        def emit_vv_group(tt):
            ps = ps_b.tile([128, CL], f32, tag="bps")
            for cb in range(CB):
                nc.tensor.matmul(
                    ps[:],
                    xT[:, cb, tt * 128 : (tt + 1) * 128],
                    wv_sb[:, cb, :],
                    start=(cb == 0),
                    stop=(cb == CB - 1),
                )
            nc.vector.tensor_copy(
                vv[tt][:, :, 0:HD], ps.rearrange("p (h d) -> p h d", d=HD)
            )

        def normalize(ups, ic, u, r0):
            rs = p_rb.tile([1, 512], f32, tag="rs", name="rs")
            nc.vector.tensor_copy(rs[:], ups[HD : HD + 1, :])
            rr = p_rb.tile([1, 512], f32, tag="rr", name="rr")
            nc.vector.reciprocal_approx_fast(rr[:], rs[:])
            rb = p_rb.tile([HD, 512], f32, tag="rb", name="rb")
            nc.gpsimd.partition_broadcast(rb[:], rr[0:1, :], channels=HD)
            nc.vector.tensor_mul(yTn[ic][r0 : r0 + HD, u, :], ups[0:HD, :], rb[:])

        def emit_pair(u, ic):
            """Attention for head pair (2u, 2u+1) over i-chunk ic. The odd
            head's P@V lags LAG j-tiles so the even head's accumulator closes
            early and its normalize chain overlaps remaining PE work."""
            co_q = u
            co_k = 4 + u
            nj = 4 * ic + 4
            lag = min(5, nj - 1)
            ups_e = ps_u.tile([HD + 1, 512], f32, tag="u", name=f"ue_{u}_{ic}")
            ups_o = ps_u.tile([HD + 1, 512], f32, tag="u", name=f"uo_{u}_{ic}")
            pts = {}

            def av(ix, ups, jt):
                m = jt % 4
                lo = 128 * m if ic == jt // 4 else 0
                nc.tensor.matmul(
                    ups[:, lo:512],
                    vv[jt][:, 2 * u + ix, :],
                    pts[jt][:, ix, lo:512],
                    start=(jt == 0),
                    stop=(jt == nj - 1),
                )

            for jt in range(nj):
                m = jt % 4
                diag = ic == jt // 4
                lo = 128 * m if diag else 0
                psg = ps_sc.tile([128, 2, 512], f32, tag="sc")
                for ix in range(2):
                    r0 = 64 * ix
                    nc.tensor.matmul(
                        psg[:, ix, lo:512],
                        qkT[(co_k, jt // 4)][r0 : r0 + 64, m * 128 : (m + 1) * 128],
                        qkT[(co_q, ic)][r0 : r0 + 64, lo:512],
                        start=True,
                        stop=True,
                    )
                pt = p_p.tile([128, 2, 512], f16, tag="p")
                pts[jt] = pt
                nc.scalar.activation(
                    pt[:, 0:2, lo:512], psg[:, 0:2, lo:512], Exp, scale=1.0 / np.sqrt(HD)
                )
                if diag:  # zero strictly-upper part of the diagonal block
                    nc.vector.tensor_mul(
                        pt[:, 0:2, lo : lo + 128],
                        pt[:, 0:2, lo : lo + 128],
                        keep2[:, 0:2, :],
                    )
                av(0, ups_e, jt)
                if jt >= lag:
                    av(1, ups_o, jt - lag)
            normalize(ups_e, ic, u, 0)
            for jt in range(nj - lag, nj):
                av(1, ups_o, jt)
            normalize(ups_o, ic, u, 64)

        def emit_proj(tn, cos, on_act=False, split_last=False):
            for co in cos:
                psp = ps_b.tile([128, 512], f32, tag="bps")
                cs = slice(co * 128, (co + 1) * 128)
                if split_last:
                    # db=3 (the last-finishing pair) contracted in two K=64
                    # halves bracketing the full-row MMs: the top half only
                    # needs the even head's normalize, and no two row-disjoint
                    # MMs are adjacent within the group (PSUM-bank rule).
                    nc.tensor.matmul(
                        psp[:], wp_sb[0:64, 3, cs], yTn[tn][0:64, 3, :],
                        start=True, stop=False,
                    )
                    for db in range(DB - 1):
                        nc.tensor.matmul(
                            psp[:], wp_sb[:, db, cs], yTn[tn][:, db, :],
                            start=False, stop=False,
                        )
                    nc.tensor.matmul(
                        psp[:], wp_sb[64:128, 3, cs], yTn[tn][64:128, 3, :],
                        start=False, stop=True,
                    )
                else:
                    for db in range(DB):
                        nc.tensor.matmul(
                            psp[:],
                            wp_sb[:, db, cs],
                            yTn[tn][:, db, :],
                            start=(db == 0),
                            stop=(db == DB - 1),
                        )
                ot = p_ost.tile([128, 512], f32, tag="ot")
                if on_act:  # tail groups: exp is done, ACT is idle, DVE is not
                    nc.scalar.activation(ot[:], psp[:], Copy)
                else:
                    nc.vector.tensor_copy(ot[:], psp[:])
                nc.sync.dma_start(
                    outT_ap[co * 128 : (co + 1) * 128, tn * 512 : (tn + 1) * 512],
                    ot[:],
                )

        # ---- B(0) up front; then C(ic) with B(ic+1)/proj(ic-1) interleaved
        for co in range(CB):
            emit_qkT_group(co, 0)
        for tt in range(4):
            emit_vv_group(tt)

        for ic in range(NIC):
            for u in range(NP):
                emit_pair(u, ic)
                if ic < NIC - 1:
                    emit_qkT_group(2 * u, ic + 1)
                    emit_qkT_group(2 * u + 1, ic + 1)
                    emit_vv_group(4 * (ic + 1) + u)
                if ic >= 1:
                    emit_proj(ic - 1, [u])
            # second half of the previous chunk's projection lands after the
            # last pair: dep-free PE filler over the final normalize chain
            if ic >= 1:
                emit_proj(ic - 1, [4, 5, 6, 7], on_act=(ic == NIC - 1))
        emit_proj(NIC - 1, range(CB), on_act=True, split_last=True)


def _build_program():
    nc = bacc.Bacc("TRN2", target_bir_lowering=False, debug=False, num_devices=N_CORES)
    aps = {
        "x": nc.dram_tensor("x", [T, C], f16, kind="ExternalInput").ap(),
        "wqk": nc.dram_tensor("wqk", [C, CB * 128], f16, kind="ExternalInput").ap(),
        "wv": nc.dram_tensor("wv", [C, CL], f16, kind="ExternalInput").ap(),
        "wp": nc.dram_tensor("wp", [CL, C], f16, kind="ExternalInput").ap(),
        "bqk": nc.dram_tensor("bqk", [CB, 128], f32, kind="ExternalInput").ap(),
        "keep2": nc.dram_tensor("keep2", [128, 2, 128], f16, kind="ExternalInput").ap(),
        "outT": nc.dram_tensor("outT", [C, T], f32, kind="ExternalOutput").ap(),
    }
    with tile.TileContext(nc) as tc:
        _emit(tc, aps)
    nc.compile()
    return nc


def get_program():
    if "nc" not in _PROG_CACHE:
        _PROG_CACHE["nc"] = _build_program()
    return _PROG_CACHE["nc"]


def _host_consts():
    j = np.arange(128)[:, None]
    i = np.arange(128)[None, :]
    keep = (j <= i).astype(np.float16)          # 1 => keep
    keep2 = np.stack([keep, keep], axis=1)      # [128, 2, 128]
    return np.ascontiguousarray(keep2)


def make_in_maps(x, W_attn, b_attn, W_proj):
    """Build the 8 per-core input maps. Core index = 2*batch + head_group."""
    keep2 = _host_consts()
    in_maps = []
    for core in range(N_CORES):
        b = core // 2
        g = core % 2
        wq = W_attn[:, g * CL : (g + 1) * CL]
        wk = W_attn[:, C + g * CL : C + (g + 1) * CL]
        wqk = np.concatenate([wq, wk], axis=1)  # [C, 1024], cols = co*128+q
        wv = W_attn[:, 2 * C + g * CL : 2 * C + (g + 1) * CL]
        bqk = np.concatenate(
            [b_attn[g * CL : (g + 1) * CL], b_attn[C + g * CL : C + (g + 1) * CL]]
        ).reshape(CB, 128)
        in_maps.append(
            {
                "x": np.ascontiguousarray(x[b]).astype(np.float16),
                "wqk": np.ascontiguousarray(wqk).astype(np.float16),
                "wv": np.ascontiguousarray(wv).astype(np.float16),
                "wp": np.ascontiguousarray(W_proj[g * CL : (g + 1) * CL, :]).astype(
                    np.float16
                ),
                "bqk": np.ascontiguousarray(bqk).astype(np.float32),
                "keep2": keep2,
            }
        )
    return in_maps


def run(x, W_attn, b_attn, W_proj, b_proj, trace=False):
    nc = get_program()
    in_maps = make_in_maps(x, W_attn, b_attn, W_proj)
    res = bass_utils.run_bass_kernel_spmd(
        nc, in_maps, core_ids=list(range(N_CORES)), trace=trace
    )
    # combine: out[b] = sum_g outT_{2b+g}^T + (bv_g @ Wp_g summed) + b_proj
    corr = b_proj.astype(np.float64).copy()
    for g in range(2):
        bv_g = b_attn[2 * C + g * CL : 2 * C + (g + 1) * CL]
        corr += bv_g.astype(np.float64) @ W_proj[g * CL : (g + 1) * CL, :].astype(
            np.float64
        )
    out = np.empty((B, T, C), np.float32)
    for b in range(B):
        acc = (
            res.results[2 * b]["outT"].T.astype(np.float64)
            + res.results[2 * b + 1]["outT"].T.astype(np.float64)
            + corr
        )
        out[b] = acc.astype(np.float32)
    return out, res


def kernel(x, W_attn, b_attn, W_proj, b_proj):
    x = np.asarray(x, np.float32)
    W_attn = np.asarray(W_attn, np.float32)
    b_attn = np.asarray(b_attn, np.float32)
    W_proj = np.asarray(W_proj, np.float32)
    b_proj = np.asarray(b_proj, np.float32)
    out, _ = run(x, W_attn, b_attn, W_proj, b_proj)
    return out


# revision 21
# speedup vs baseline: 1.0038x; 1.0038x over previous
"""Causal self-attention (B=4, T=2048, C=1024, H=16) on 8 Trainium2 NeuronCores.

Core index = 2*batch + head_group: each core owns one batch element and 8 of
the 16 heads (tensor-parallel split of c_attn output dim / c_proj input dim).
Each core emits a partial projection out^T [C, T]; the host sums the two
head-group partials per batch and adds the bias terms.

v2 (vs v1 baseline 358us):
  * Head-PAIR processing: even local head lives on SBUF partitions 0-63,
    odd head on 64-127. The two K=64 scores matmuls of a pair are emitted
    back-to-back with inferred tile_position (0,0)/(64,0) -> they run
    CONCURRENTLY in the PE array (row-group tiling), and their LDWEIGHTS
    cross-hide against the other row group's matmul.
  * No negI mask matmuls: causal masking is a post-exp DVE multiply of the
    128-wide diagonal block only (PE -21us, DVE +16us).
  * qkv projection (phase B), attention (phase C) and out-projection are
    interleaved in emission order so the Tile scheduler can fill PE stalls
    (C is ACT-exp-bound in bursts) with independent B/proj matmuls.
  * x DMA-transposes split across the two HWDGE queues (sync+scalar) and
    issued first: PE starts ~6us in instead of ~24us.
  * normalize reads U'/rowsum directly from PSUM (no staging copies).

fp16 datapath (fp32 PSUM accumulation everywhere, fp32 softmax denominator).

Per-core pipeline per head pair u (heads 2u, 2u+1), per 512-wide i-chunk ic:
  S^T[j, i] = k_h^T q_h  for both heads -> one psum tile [128, 2, 512]
  P = exp(S^T / 8)       (one ACT op per (jt, ic) sub, both heads)
  diagonal j-tile: P *= keep (DVE, precomputed lower-tri mask)
  U'^T [65, i] (+)= [v_h|1]^T P_h^T  over j-tiles (ones col => rowsum row 64)
  yT[u-block, i] = U'[0:64] * bcast(1/rowsum)
out^T = W_p^T yT -> fp32 psum -> DVE copy -> DMA
"""

import numpy as np

import concourse.bass as bass
import concourse.mybir as mybir
import concourse.tile as tile
from concourse import bacc, bass_utils

B, T, C, H = 4, 2048, 1024, 16
HD = C // H          # 64 head dim
N_CORES = 8
HG = H // 2          # 8 heads per core
CL = HG * HD         # 512 local width of q/k/v
TT = T // 128        # 16 t-tiles
CB = C // 128        # 8 c-tiles
DB = CL // 128       # 4 local-hd tiles
NIC = T // 512       # i-chunks (4)
NP = HG // 2         # head pairs per core (4)

f32 = mybir.dt.float32
f16 = mybir.dt.float16

_PROG_CACHE = {}


def _emit(tc, aps):
    nc = tc.nc
    Exp = mybir.ActivationFunctionType.Exp
    Copy = mybir.ActivationFunctionType.Copy

    x_ap = aps["x"]
    wqk_ap = aps["wqk"]
    wv_ap = aps["wv"]
    wp_ap = aps["wp"]
    bqk_ap = aps["bqk"]
    keep2_ap = aps["keep2"]
    outT_ap = aps["outT"]

    from contextlib import ExitStack

    with ExitStack() as outer:
        const = outer.enter_context(tc.tile_pool(name="const", bufs=1))
        p_xT = outer.enter_context(tc.tile_pool(name="xT", bufs=1))
        p_qkT = outer.enter_context(tc.tile_pool(name="qkT", bufs=1))
        p_v = outer.enter_context(tc.tile_pool(name="vv", bufs=1))
        p_yT = outer.enter_context(tc.tile_pool(name="yT", bufs=1))
        p_w = outer.enter_context(tc.tile_pool(name="wsb", bufs=1))
        p_p = outer.enter_context(tc.tile_pool(name="pp", bufs=8))
        p_rb = outer.enter_context(tc.tile_pool(name="rb", bufs=3))
        p_ost = outer.enter_context(tc.tile_pool(name="ost", bufs=4))
        ps_b = outer.enter_context(tc.tile_pool(name="ps_b", bufs=2, space="PSUM"))
        ps_sc = outer.enter_context(tc.tile_pool(name="ps_sc", bufs=2, space="PSUM"))
        ps_u = outer.enter_context(tc.tile_pool(name="ps_u", bufs=2, space="PSUM"))

        # ---- DMAs: x transposes are the PE critical path -> they own both
        # HWDGE queues (sync+scalar), with t-chunk 0 split across the two so
        # its descriptor GENERATION (~6us each) runs in parallel. All weight
        # loads go through the gpsimd SWDGE queue (cheap generation).
        # x transposes: XBAR transfer is fast (~14ns/16x128 tile); the cost is
        # per-instruction DGE latency, which pipelines across the two HWDGE
        # queues. Small first pieces unblock B(0) early.
        # Warm-up tile memset leads the otherwise-idle gpsimd queue.
        warm = const.tile([128, 512], f16)
        nc.gpsimd.memset(warm[:], 0.0)
        # B(0) needs exactly chunk0-transposed + the wqk q-half (2MB): those
        # two lead their queues; ALL other input DMAs queue strictly behind
        # them so they don't steal DMA-fabric bandwidth from the critical
        # pair. XBAR transposes serialize FIFO across queues -> all on scalar.
        xT = p_xT.tile([128, CB, T], f16)
        nc.scalar.dma_start_transpose(xT[:, :, 0:256], x_ap[0:256, :])
        nc.scalar.dma_start_transpose(xT[:, :, 256:512], x_ap[256:512, :])
        nc.scalar.dma_start_transpose(xT[:, :, 512:1024], x_ap[512:1024, :])
        nc.scalar.dma_start_transpose(xT[:, :, 1024:1536], x_ap[1024:1536, :])
        nc.scalar.dma_start_transpose(xT[:, :, 1536:2048], x_ap[1536:2048, :])
        wqk_sb = p_w.tile([128, CB, CB * 128], f16)  # [c-part, cb, co*128+q]
        nc.sync.dma_start(
            wqk_sb[:, :, 0 : 4 * 128],
            wqk_ap[:, 0 : 4 * 128].rearrange("(cb p) n -> p cb n", p=128),
        )
        bqk = const.tile([128, CB], f32)
        nc.sync.dma_start(bqk[:], bqk_ap.rearrange("co p -> p co"))
        nc.sync.dma_start(
            wqk_sb[:, :, 4 * 128 : 8 * 128],
            wqk_ap[:, 4 * 128 : 8 * 128].rearrange("(cb p) n -> p cb n", p=128),
        )
        wv_sb = p_w.tile([128, CB, CL], f16)
        nc.sync.dma_start(wv_sb[:], wv_ap.rearrange("(cb p) n -> p cb n", p=128))
        wp_sb = p_w.tile([128, DB, C], f16)
        nc.sync.dma_start(wp_sb[:], wp_ap.rearrange("(db p) c -> p db c", p=128))
        keep2 = const.tile([128, 2, 128], f16)   # keep[j, ix, i] = (j <= i)
        nc.gpsimd.dma_start(keep2[:], keep2_ap)

        # ---- PE warm-up: dep-free matmuls on the zeroed tile keep the PE
        # busy from t~6us so HAM un-throttles to 2.4 GHz before real work
        # arrives, and the array never sits cold waiting for the first chunk.
        wups = ps_u.tile([HD + 1, 512], f32, tag="u", name="wups")
        for _ in range(18):
            nc.tensor.matmul(
                wups[:], warm[:, 0 : HD + 1], warm[:], start=True, stop=True
            )

        # per-(co, tn) qkT tiles, per-jt v' tiles, per-tn yT tiles
        qkT = {}
        for co in range(CB):
            for tn in range(NIC):
                qkT[(co, tn)] = p_qkT.tile(
                    [128, 512], f16, tag=f"qkT_{co}_{tn}", name=f"qkT_{co}_{tn}"
                )
        vv = {}
        for jt in range(TT):
            vv[jt] = p_v.tile([128, HG, HD + 1], f16, tag=f"vv_{jt}", name=f"vv_{jt}")
            nc.vector.memset(vv[jt][:, :, HD : HD + 1], 1.0)
        yTn = {}
        for tn in range(NIC):
            yTn[tn] = p_yT.tile([128, DB, 512], f16, tag=f"yT_{tn}", name=f"yT_{tn}")

        def emit_qkT_group(co, tn):
            ps = ps_b.tile([128, 512], f32, tag="bps")
            for cb in range(CB):
                nc.tensor.matmul(
                    ps[:],
                    wqk_sb[:, cb, co * 128 : (co + 1) * 128],
                    xT[:, cb, tn * 512 : (tn + 1) * 512],
                    start=(cb == 0),
                    stop=(cb == CB - 1),
                )
            nc.vector.tensor_scalar_add(qkT[(co, tn)][:], ps[:], bqk[:, co : co + 1])

        def emit_vv_group(tt):
            ps = ps_b.tile([128, CL], f32, tag="bps")
            for cb in range(CB):
                nc.tensor.matmul(
                    ps[:],
                    xT[:, cb, tt * 128 : (tt + 1) * 128],
                    wv_sb[:, cb, :],
                    start=(cb == 0),
                    stop=(cb == CB - 1),
                )
            nc.vector.tensor_copy(
                vv[tt][:, :, 0:HD], ps.rearrange("p (h d) -> p h d", d=HD)
            )

        def normalize(ups, ic, u, r0):
            rs = p_rb.tile([1, 512], f32, tag="rs", name="rs")
            nc.vector.tensor_copy(rs[:], ups[HD : HD + 1, :])
            rr = p_rb.tile([1, 512], f32, tag="rr", name="rr")
            nc.vector.reciprocal_approx_fast(rr[:], rs[:])
            rb = p_rb.tile([HD, 512], f32, tag="rb", name="rb")
            nc.gpsimd.partition_broadcast(rb[:], rr[0:1, :], channels=HD)
            nc.vector.tensor_mul(yTn[ic][r0 : r0 + HD, u, :], ups[0:HD, :], rb[:])

        def emit_pair(u, ic):
            """Attention for head pair (2u, 2u+1) over i-chunk ic. The odd
            head's P@V lags LAG j-tiles so the even head's accumulator closes
            early and its normalize chain overlaps remaining PE work."""
            co_q = u
            co_k = 4 + u
            nj = 4 * ic + 4
            lag = min(5, nj - 1)
            ups_e = ps_u.tile([HD + 1, 512], f32, tag="u", name=f"ue_{u}_{ic}")
            ups_o = ps_u.tile([HD + 1, 512], f32, tag="u", name=f"uo_{u}_{ic}")
            pts = {}

            def av(ix, ups, jt):
                m = jt % 4
                lo = 128 * m if ic == jt // 4 else 0
                nc.tensor.matmul(
                    ups[:, lo:512],
                    vv[jt][:, 2 * u + ix, :],
                    pts[jt][:, ix, lo:512],
                    start=(jt == 0),
                    stop=(jt == nj - 1),
                )

            for jt in range(nj):
                m = jt % 4
                diag = ic == jt // 4
                lo = 128 * m if diag else 0
                psg = ps_sc.tile([128, 2, 512], f32, tag="sc")
                for ix in range(2):
                    r0 = 64 * ix
                    nc.tensor.matmul(
                        psg[:, ix, lo:512],
                        qkT[(co_k, jt // 4)][r0 : r0 + 64, m * 128 : (m + 1) * 128],
                        qkT[(co_q, ic)][r0 : r0 + 64, lo:512],
                        start=True,
                        stop=True,
                    )
                pt = p_p.tile([128, 2, 512], f16, tag="p")
                pts[jt] = pt
                nc.scalar.activation(
                    pt[:, 0:2, lo:512], psg[:, 0:2, lo:512], Exp, scale=1.0 / np.sqrt(HD)
                )
                if diag:  # zero strictly-upper part of the diagonal block
                    nc.vector.tensor_mul(
                        pt[:, 0:2, lo : lo + 128],
                        pt[:, 0:2, lo : lo + 128],
                        keep2[:, 0:2, :],
                    )
                av(0, ups_e, jt)
                if jt >= lag:
                    av(1, ups_o, jt - lag)
            normalize(ups_e, ic, u, 0)
            for jt in range(nj - lag, nj):
                av(1, ups_o, jt)
            normalize(ups_o, ic, u, 64)

        def emit_proj(tn, cos, on_act=False, split_last=False):
            for co in cos:
                psp = ps_b.tile([128, 512], f32, tag="bps")
                cs = slice(co * 128, (co + 1) * 128)
                if split_last:
                    # db=3 (the last-finishing pair) contracted in two K=64
                    # halves bracketing the full-row MMs: the top half only
                    # needs the even head's normalize, and no two row-disjoint
                    # MMs are adjacent within the group (PSUM-bank rule).
                    nc.tensor.matmul(
                        psp[:], wp_sb[0:64, 3, cs], yTn[tn][0:64, 3, :],
                        start=True, stop=False,
                    )
                    for db in range(DB - 1):
                        nc.tensor.matmul(
                            psp[:], wp_sb[:, db, cs], yTn[tn][:, db, :],
                            start=False, stop=False,
                        )
                    nc.tensor.matmul(
                        psp[:], wp_sb[64:128, 3, cs], yTn[tn][64:128, 3, :],
                        start=False, stop=True,
                    )
                else:
                    for db in range(DB):
                        nc.tensor.matmul(
                            psp[:],
                            wp_sb[:, db, cs],
                            yTn[tn][:, db, :],
                            start=(db == 0),
                            stop=(db == DB - 1),
                        )
                ot = p_ost.tile([128, 512], f32, tag="ot")
                if on_act:  # tail groups: exp is done, ACT is idle, DVE is not
                    nc.scalar.activation(ot[:], psp[:], Copy)
                else:
                    nc.vector.tensor_copy(ot[:], psp[:])
                nc.sync.dma_start(
                    outT_ap[co * 128 : (co + 1) * 128, tn * 512 : (tn + 1) * 512],
                    ot[:],
                )

        # ---- B(0) up front; then C(ic) with B(ic+1)/proj(ic-1) interleaved
        for co in range(CB):
            emit_qkT_group(co, 0)
        for tt in range(4):
            emit_vv_group(tt)

        for ic in range(NIC):
            for u in range(NP):
                emit_pair(u, ic)
                if ic < NIC - 1:
                    emit_qkT_group(2 * u, ic + 1)
                    emit_qkT_group(2 * u + 1, ic + 1)
                    emit_vv_group(4 * (ic + 1) + u)
                if ic >= 1:
                    emit_proj(ic - 1, [u])
            # second half of the previous chunk's projection lands after the
            # last pair: dep-free PE filler over the final normalize chain
            if ic >= 1:
                emit_proj(ic - 1, [4, 5, 6, 7], on_act=(ic == NIC - 1))
        emit_proj(NIC - 1, range(CB), on_act=True, split_last=True)


def _build_program():
    nc = bacc.Bacc("TRN2", target_bir_lowering=False, debug=False, num_devices=N_CORES)
    aps = {
        "x": nc.dram_tensor("x", [T, C], f16, kind="ExternalInput").ap(),
        "wqk": nc.dram_tensor("wqk", [C, CB * 128], f16, kind="ExternalInput").ap(),
        "wv": nc.dram_tensor("wv", [C, CL], f16, kind="ExternalInput").ap(),
        "wp": nc.dram_tensor("wp", [CL, C], f16, kind="ExternalInput").ap(),
        "bqk": nc.dram_tensor("bqk", [CB, 128], f32, kind="ExternalInput").ap(),
        "keep2": nc.dram_tensor("keep2", [128, 2, 128], f16, kind="ExternalInput").ap(),
        "outT": nc.dram_tensor("outT", [C, T], f32, kind="ExternalOutput").ap(),
    }
    with tile.TileContext(nc) as tc:
        _emit(tc, aps)
    nc.compile()
    return nc


def get_program():
    if "nc" not in _PROG_CACHE:
        _PROG_CACHE["nc"] = _build_program()
    return _PROG_CACHE["nc"]


def _host_consts():
    j = np.arange(128)[:, None]
    i = np.arange(128)[None, :]
    keep = (j <= i).astype(np.float16)          # 1 => keep
    keep2 = np.stack([keep, keep], axis=1)      # [128, 2, 128]
    return np.ascontiguousarray(keep2)


def make_in_maps(x, W_attn, b_attn, W_proj):
    """Build the 8 per-core input maps. Core index = 2*batch + head_group."""
    keep2 = _host_consts()
    in_maps = []
    for core in range(N_CORES):
        b = core // 2
        g = core % 2
        wq = W_attn[:, g * CL : (g + 1) * CL]
        wk = W_attn[:, C + g * CL : C + (g + 1) * CL]
        wqk = np.concatenate([wq, wk], axis=1)  # [C, 1024], cols = co*128+q
        wv = W_attn[:, 2 * C + g * CL : 2 * C + (g + 1) * CL]
        bqk = np.concatenate(
            [b_attn[g * CL : (g + 1) * CL], b_attn[C + g * CL : C + (g + 1) * CL]]
        ).reshape(CB, 128)
        in_maps.append(
            {
                "x": np.ascontiguousarray(x[b]).astype(np.float16),
                "wqk": np.ascontiguousarray(wqk).astype(np.float16),
                "wv": np.ascontiguousarray(wv).astype(np.float16),
                "wp": np.ascontiguousarray(W_proj[g * CL : (g + 1) * CL, :]).astype(
                    np.float16
                ),
                "bqk": np.ascontiguousarray(bqk).astype(np.float32),
                "keep2": keep2,
            }
        )
    return in_maps


def run(x, W_attn, b_attn, W_proj, b_proj, trace=False):
    nc = get_program()
    in_maps = make_in_maps(x, W_attn, b_attn, W_proj)
    res = bass_utils.run_bass_kernel_spmd(
        nc, in_maps, core_ids=list(range(N_CORES)), trace=trace
    )
    # combine: out[b] = sum_g outT_{2b+g}^T + (bv_g @ Wp_g summed) + b_proj
    corr = b_proj.astype(np.float64).copy()
    for g in range(2):
        bv_g = b_attn[2 * C + g * CL : 2 * C + (g + 1) * CL]
        corr += bv_g.astype(np.float64) @ W_proj[g * CL : (g + 1) * CL, :].astype(
            np.float64
        )
    out = np.empty((B, T, C), np.float32)
    for b in range(B):
        acc = (
            res.results[2 * b]["outT"].T.astype(np.float64)
            + res.results[2 * b + 1]["outT"].T.astype(np.float64)
            + corr
        )
        out[b] = acc.astype(np.float32)
    return out, res


def kernel(x, W_attn, b_attn, W_proj, b_proj):
    x = np.asarray(x, np.float32)
    W_attn = np.asarray(W_attn, np.float32)
    b_attn = np.asarray(b_attn, np.float32)
    W_proj = np.asarray(W_proj, np.float32)
    b_proj = np.asarray(b_proj, np.float32)
    out, _ = run(x, W_attn, b_attn, W_proj, b_proj)
    return out


# revision 23
# speedup vs baseline: 1.0239x; 1.0201x over previous
"""Causal self-attention (B=4, T=2048, C=1024, H=16) on 8 Trainium2 NeuronCores.

Core index = 2*batch + head_group: each core owns one batch element and 8 of
the 16 heads (tensor-parallel split of c_attn output dim / c_proj input dim).
Each core emits a partial projection out^T [C, T]; the host sums the two
head-group partials per batch and adds the bias terms.

v2 (vs v1 baseline 358us):
  * Head-PAIR processing: even local head lives on SBUF partitions 0-63,
    odd head on 64-127. The two K=64 scores matmuls of a pair are emitted
    back-to-back with inferred tile_position (0,0)/(64,0) -> they run
    CONCURRENTLY in the PE array (row-group tiling), and their LDWEIGHTS
    cross-hide against the other row group's matmul.
  * No negI mask matmuls: causal masking is a post-exp DVE multiply of the
    128-wide diagonal block only (PE -21us, DVE +16us).
  * qkv projection (phase B), attention (phase C) and out-projection are
    interleaved in emission order so the Tile scheduler can fill PE stalls
    (C is ACT-exp-bound in bursts) with independent B/proj matmuls.
  * x DMA-transposes split across the two HWDGE queues (sync+scalar) and
    issued first: PE starts ~6us in instead of ~24us.
  * normalize reads U'/rowsum directly from PSUM (no staging copies).

fp16 datapath (fp32 PSUM accumulation everywhere, fp32 softmax denominator).

Per-core pipeline per head pair u (heads 2u, 2u+1), per 512-wide i-chunk ic:
  S^T[j, i] = k_h^T q_h  for both heads -> one psum tile [128, 2, 512]
  P = exp(S^T / 8)       (one ACT op per (jt, ic) sub, both heads)
  diagonal j-tile: P *= keep (DVE, precomputed lower-tri mask)
  U'^T [65, i] (+)= [v_h|1]^T P_h^T  over j-tiles (ones col => rowsum row 64)
  yT[u-block, i] = U'[0:64] * bcast(1/rowsum)
out^T = W_p^T yT -> fp32 psum -> DVE copy -> DMA
"""

import numpy as np

import concourse.bass as bass
import concourse.mybir as mybir
import concourse.tile as tile
from concourse import bacc, bass_utils

B, T, C, H = 4, 2048, 1024, 16
HD = C // H          # 64 head dim
N_CORES = 8
HG = H // 2          # 8 heads per core
CL = HG * HD         # 512 local width of q/k/v
TT = T // 128        # 16 t-tiles
CB = C // 128        # 8 c-tiles
DB = CL // 128       # 4 local-hd tiles
NIC = T // 512       # i-chunks (4)
NP = HG // 2         # head pairs per core (4)

f32 = mybir.dt.float32
f16 = mybir.dt.float16

_PROG_CACHE = {}


def _emit(tc, aps):
    nc = tc.nc
    Exp = mybir.ActivationFunctionType.Exp
    Copy = mybir.ActivationFunctionType.Copy

    x_ap = aps["x"]
    wqk_ap = aps["wqk"]
    wv_ap = aps["wv"]
    wp_ap = aps["wp"]
    bqk_ap = aps["bqk"]
    keep2_ap = aps["keep2"]
    outT_ap = aps["outT"]

    from contextlib import ExitStack

    with ExitStack() as outer:
        const = outer.enter_context(tc.tile_pool(name="const", bufs=1))
        p_xT = outer.enter_context(tc.tile_pool(name="xT", bufs=1))
        p_qkT = outer.enter_context(tc.tile_pool(name="qkT", bufs=1))
        p_v = outer.enter_context(tc.tile_pool(name="vv", bufs=1))
        p_yT = outer.enter_context(tc.tile_pool(name="yT", bufs=1))
        p_w = outer.enter_context(tc.tile_pool(name="wsb", bufs=1))
        p_p = outer.enter_context(tc.tile_pool(name="pp", bufs=8))
        p_rb = outer.enter_context(tc.tile_pool(name="rb", bufs=3))
        p_ost = outer.enter_context(tc.tile_pool(name="ost", bufs=4))
        ps_b = outer.enter_context(tc.tile_pool(name="ps_b", bufs=2, space="PSUM"))
        ps_sc = outer.enter_context(tc.tile_pool(name="ps_sc", bufs=2, space="PSUM"))
        ps_u = outer.enter_context(tc.tile_pool(name="ps_u", bufs=2, space="PSUM"))

        # ---- DMAs: x transposes are the PE critical path -> they own both
        # HWDGE queues (sync+scalar), with t-chunk 0 split across the two so
        # its descriptor GENERATION (~6us each) runs in parallel. All weight
        # loads go through the gpsimd SWDGE queue (cheap generation).
        # x transposes: XBAR transfer is fast (~14ns/16x128 tile); the cost is
        # per-instruction DGE latency, which pipelines across the two HWDGE
        # queues. Small first pieces unblock B(0) early.
        # Warm-up tile memset leads the otherwise-idle gpsimd queue.
        warm = const.tile([128, 512], f16)
        nc.gpsimd.memset(warm[:], 0.0)
        # B(0) needs exactly chunk0-transposed + the wqk q-half (2MB): those
        # two lead their queues; ALL other input DMAs queue strictly behind
        # them so they don't steal DMA-fabric bandwidth from the critical
        # pair. XBAR transposes serialize FIFO across queues -> all on scalar.
        xT = p_xT.tile([128, CB, T], f16)
        nc.scalar.dma_start_transpose(xT[:, :, 0:256], x_ap[0:256, :])
        nc.scalar.dma_start_transpose(xT[:, :, 256:512], x_ap[256:512, :])
        nc.scalar.dma_start_transpose(xT[:, :, 512:1024], x_ap[512:1024, :])
        nc.scalar.dma_start_transpose(xT[:, :, 1024:1536], x_ap[1024:1536, :])
        nc.scalar.dma_start_transpose(xT[:, :, 1536:2048], x_ap[1536:2048, :])
        wqk_sb = p_w.tile([128, CB, CB * 128], f16)  # [c-part, cb, co*128+q]
        nc.sync.dma_start(
            wqk_sb[:, :, 0 : 4 * 128],
            wqk_ap[:, 0 : 4 * 128].rearrange("(cb p) n -> p cb n", p=128),
        )
        bqk = const.tile([128, CB], f32)
        nc.sync.dma_start(bqk[:], bqk_ap.rearrange("co p -> p co"))
        # k-half on the gpsimd queue: runs parallel to sync's q-half, so the
        # first scores (which need k) aren't gated behind wv/wp.
        nc.gpsimd.dma_start(
            wqk_sb[:, :, 4 * 128 : 8 * 128],
            wqk_ap[:, 4 * 128 : 8 * 128].rearrange("(cb p) n -> p cb n", p=128),
        )
        keep2 = const.tile([128, 2, 128], f16)   # keep[j, ix, i] = (j <= i)
        nc.gpsimd.dma_start(keep2[:], keep2_ap)
        wv_sb = p_w.tile([128, CB, CL], f16)
        nc.sync.dma_start(wv_sb[:], wv_ap.rearrange("(cb p) n -> p cb n", p=128))
        wp_sb = p_w.tile([128, DB, C], f16)
        nc.sync.dma_start(wp_sb[:], wp_ap.rearrange("(db p) c -> p db c", p=128))

        # ---- PE warm-up: dep-free matmuls on the zeroed tile keep the PE
        # busy from t~6us so HAM un-throttles to 2.4 GHz before real work
        # arrives, and the array never sits cold waiting for the first chunk.
        wups = ps_u.tile([HD + 1, 512], f32, tag="u", name="wups")
        for _ in range(26):
            nc.tensor.matmul(
                wups[:], warm[:, 0 : HD + 1], warm[:], start=True, stop=True
            )

        # per-(co, tn) qkT tiles, per-jt v' tiles, per-tn yT tiles
        qkT = {}
        for co in range(CB):
            for tn in range(NIC):
                qkT[(co, tn)] = p_qkT.tile(
                    [128, 512], f16, tag=f"qkT_{co}_{tn}", name=f"qkT_{co}_{tn}"
                )
        vv = {}
        for jt in range(TT):
            vv[jt] = p_v.tile([128, HG, HD + 1], f16, tag=f"vv_{jt}", name=f"vv_{jt}")
            nc.vector.memset(vv[jt][:, :, HD : HD + 1], 1.0)
        yTn = {}
        for tn in range(NIC):
            yTn[tn] = p_yT.tile([128, DB, 512], f16, tag=f"yT_{tn}", name=f"yT_{tn}")

        def emit_qkT_group(co, tn):
            ps = ps_b.tile([128, 512], f32, tag="bps")
            for cb in range(CB):
                nc.tensor.matmul(
                    ps[:],
                    wqk_sb[:, cb, co * 128 : (co + 1) * 128],
                    xT[:, cb, tn * 512 : (tn + 1) * 512],
                    start=(cb == 0),
                    stop=(cb == CB - 1),
                )
            nc.vector.tensor_scalar_add(qkT[(co, tn)][:], ps[:], bqk[:, co : co + 1])

        def emit_vv_group(tt):
            ps = ps_b.tile([128, CL], f32, tag="bps")
            for cb in range(CB):
                nc.tensor.matmul(
                    ps[:],
                    xT[:, cb, tt * 128 : (tt + 1) * 128],
                    wv_sb[:, cb, :],
                    start=(cb == 0),
                    stop=(cb == CB - 1),
                )
            nc.vector.tensor_copy(
                vv[tt][:, :, 0:HD], ps.rearrange("p (h d) -> p h d", d=HD)
            )

        def normalize(ups, ic, u, r0):
            rs = p_rb.tile([1, 512], f32, tag="rs", name="rs")
            nc.vector.tensor_copy(rs[:], ups[HD : HD + 1, :])
            rr = p_rb.tile([1, 512], f32, tag="rr", name="rr")
            nc.vector.reciprocal_approx_fast(rr[:], rs[:])
            rb = p_rb.tile([HD, 512], f32, tag="rb", name="rb")
            nc.gpsimd.partition_broadcast(rb[:], rr[0:1, :], channels=HD)
            nc.vector.tensor_mul(yTn[ic][r0 : r0 + HD, u, :], ups[0:HD, :], rb[:])

        def emit_pair(u, ic):
            """Attention for head pair (2u, 2u+1) over i-chunk ic. The odd
            head's P@V lags LAG j-tiles so the even head's accumulator closes
            early and its normalize chain overlaps remaining PE work."""
            co_q = u
            co_k = 4 + u
            nj = 4 * ic + 4
            lag = min(5, nj - 1)
            ups_e = ps_u.tile([HD + 1, 512], f32, tag="u", name=f"ue_{u}_{ic}")
            ups_o = ps_u.tile([HD + 1, 512], f32, tag="u", name=f"uo_{u}_{ic}")
            pts = {}

            def av(ix, ups, jt):
                m = jt % 4
                lo = 128 * m if ic == jt // 4 else 0
                nc.tensor.matmul(
                    ups[:, lo:512],
                    vv[jt][:, 2 * u + ix, :],
                    pts[jt][:, ix, lo:512],
                    start=(jt == 0),
                    stop=(jt == nj - 1),
                )

            for jt in range(nj):
                m = jt % 4
                diag = ic == jt // 4
                lo = 128 * m if diag else 0
                psg = ps_sc.tile([128, 2, 512], f32, tag="sc")
                for ix in range(2):
                    r0 = 64 * ix
                    nc.tensor.matmul(
                        psg[:, ix, lo:512],
                        qkT[(co_k, jt // 4)][r0 : r0 + 64, m * 128 : (m + 1) * 128],
                        qkT[(co_q, ic)][r0 : r0 + 64, lo:512],
                        start=True,
                        stop=True,
                    )
                pt = p_p.tile([128, 2, 512], f16, tag="p")
                pts[jt] = pt
                nc.scalar.activation(
                    pt[:, 0:2, lo:512], psg[:, 0:2, lo:512], Exp, scale=1.0 / np.sqrt(HD)
                )
                if diag:  # zero strictly-upper part of the diagonal block
                    nc.vector.tensor_mul(
                        pt[:, 0:2, lo : lo + 128],
                        pt[:, 0:2, lo : lo + 128],
                        keep2[:, 0:2, :],
                    )
                av(0, ups_e, jt)
                if jt >= lag:
                    av(1, ups_o, jt - lag)
            normalize(ups_e, ic, u, 0)
            for jt in range(nj - lag, nj):
                av(1, ups_o, jt)
            normalize(ups_o, ic, u, 64)

        def emit_proj(tn, cos, on_act=False, split_last=False):
            for co in cos:
                psp = ps_b.tile([128, 512], f32, tag="bps")
                cs = slice(co * 128, (co + 1) * 128)
                if split_last:
                    # db=3 (the last-finishing pair) contracted in two K=64
                    # halves bracketing the full-row MMs: the top half only
                    # needs the even head's normalize, and no two row-disjoint
                    # MMs are adjacent within the group (PSUM-bank rule).
                    nc.tensor.matmul(
                        psp[:], wp_sb[0:64, 3, cs], yTn[tn][0:64, 3, :],
                        start=True, stop=False,
                    )
                    for db in range(DB - 1):
                        nc.tensor.matmul(
                            psp[:], wp_sb[:, db, cs], yTn[tn][:, db, :],
                            start=False, stop=False,
                        )
                    nc.tensor.matmul(
                        psp[:], wp_sb[64:128, 3, cs], yTn[tn][64:128, 3, :],
                        start=False, stop=True,
                    )
                else:
                    for db in range(DB):
                        nc.tensor.matmul(
                            psp[:],
                            wp_sb[:, db, cs],
                            yTn[tn][:, db, :],
                            start=(db == 0),
                            stop=(db == DB - 1),
                        )
                ot = p_ost.tile([128, 512], f32, tag="ot")
                if on_act:  # tail groups: exp is done, ACT is idle, DVE is not
                    nc.scalar.activation(ot[:], psp[:], Copy)
                else:
                    nc.vector.tensor_copy(ot[:], psp[:])
                nc.sync.dma_start(
                    outT_ap[co * 128 : (co + 1) * 128, tn * 512 : (tn + 1) * 512],
                    ot[:],
                )

        # ---- B(0) up front; then C(ic) with B(ic+1)/proj(ic-1) interleaved
        for co in range(CB):
            emit_qkT_group(co, 0)
        for tt in range(4):
            emit_vv_group(tt)

        for ic in range(NIC):
            for u in range(NP):
                emit_pair(u, ic)
                if ic < NIC - 1:
                    emit_qkT_group(2 * u, ic + 1)
                    emit_qkT_group(2 * u + 1, ic + 1)
                    emit_vv_group(4 * (ic + 1) + u)
                if ic >= 1:
                    emit_proj(ic - 1, [u])
            # second half of the previous chunk's projection lands after the
            # last pair: dep-free PE filler over the final normalize chain
            if ic >= 1:
                emit_proj(ic - 1, [4, 5, 6, 7], on_act=(ic == NIC - 1))
        emit_proj(NIC - 1, range(CB), on_act=True, split_last=True)


def _build_program():
    nc = bacc.Bacc("TRN2", target_bir_lowering=False, debug=False, num_devices=N_CORES)
    aps = {
        "x": nc.dram_tensor("x", [T, C], f16, kind="ExternalInput").ap(),
        "wqk": nc.dram_tensor("wqk", [C, CB * 128], f16, kind="ExternalInput").ap(),
        "wv": nc.dram_tensor("wv", [C, CL], f16, kind="ExternalInput").ap(),
        "wp": nc.dram_tensor("wp", [CL, C], f16, kind="ExternalInput").ap(),
        "bqk": nc.dram_tensor("bqk", [CB, 128], f32, kind="ExternalInput").ap(),
        "keep2": nc.dram_tensor("keep2", [128, 2, 128], f16, kind="ExternalInput").ap(),
        "outT": nc.dram_tensor("outT", [C, T], f32, kind="ExternalOutput").ap(),
    }
    with tile.TileContext(nc) as tc:
        _emit(tc, aps)
    nc.compile()
    return nc


def get_program():
    if "nc" not in _PROG_CACHE:
        _PROG_CACHE["nc"] = _build_program()
    return _PROG_CACHE["nc"]


def _host_consts():
    j = np.arange(128)[:, None]
    i = np.arange(128)[None, :]
    keep = (j <= i).astype(np.float16)          # 1 => keep
    keep2 = np.stack([keep, keep], axis=1)      # [128, 2, 128]
    return np.ascontiguousarray(keep2)


def make_in_maps(x, W_attn, b_attn, W_proj):
    """Build the 8 per-core input maps. Core index = 2*batch + head_group."""
    keep2 = _host_consts()
    in_maps = []
    for core in range(N_CORES):
        b = core // 2
        g = core % 2
        wq = W_attn[:, g * CL : (g + 1) * CL]
        wk = W_attn[:, C + g * CL : C + (g + 1) * CL]
        wqk = np.concatenate([wq, wk], axis=1)  # [C, 1024], cols = co*128+q
        wv = W_attn[:, 2 * C + g * CL : 2 * C + (g + 1) * CL]
        bqk = np.concatenate(
            [b_attn[g * CL : (g + 1) * CL], b_attn[C + g * CL : C + (g + 1) * CL]]
        ).reshape(CB, 128)
        in_maps.append(
            {
                "x": np.ascontiguousarray(x[b]).astype(np.float16),
                "wqk": np.ascontiguousarray(wqk).astype(np.float16),
                "wv": np.ascontiguousarray(wv).astype(np.float16),
                "wp": np.ascontiguousarray(W_proj[g * CL : (g + 1) * CL, :]).astype(
                    np.float16
                ),
                "bqk": np.ascontiguousarray(bqk).astype(np.float32),
                "keep2": keep2,
            }
        )
    return in_maps


def run(x, W_attn, b_attn, W_proj, b_proj, trace=False):
    nc = get_program()
    in_maps = make_in_maps(x, W_attn, b_attn, W_proj)
    res = bass_utils.run_bass_kernel_spmd(
        nc, in_maps, core_ids=list(range(N_CORES)), trace=trace
    )
    # combine: out[b] = sum_g outT_{2b+g}^T + (bv_g @ Wp_g summed) + b_proj
    corr = b_proj.astype(np.float64).copy()
    for g in range(2):
        bv_g = b_attn[2 * C + g * CL : 2 * C + (g + 1) * CL]
        corr += bv_g.astype(np.float64) @ W_proj[g * CL : (g + 1) * CL, :].astype(
            np.float64
        )
    out = np.empty((B, T, C), np.float32)
    for b in range(B):
        acc = (
            res.results[2 * b]["outT"].T.astype(np.float64)
            + res.results[2 * b + 1]["outT"].T.astype(np.float64)
            + corr
        )
        out[b] = acc.astype(np.float32)
    return out, res


def kernel(x, W_attn, b_attn, W_proj, b_proj):
    x = np.asarray(x, np.float32)
    W_attn = np.asarray(W_attn, np.float32)
    b_attn = np.asarray(b_attn, np.float32)
    W_proj = np.asarray(W_proj, np.float32)
    b_proj = np.asarray(b_proj, np.float32)
    out, _ = run(x, W_attn, b_attn, W_proj, b_proj)
    return out


# revision 25
# speedup vs baseline: 1.0291x; 1.0051x over previous
"""Causal self-attention (B=4, T=2048, C=1024, H=16) on 8 Trainium2 NeuronCores.

Core index = 2*batch + head_group: each core owns one batch element and 8 of
the 16 heads (tensor-parallel split of c_attn output dim / c_proj input dim).
Each core emits a partial projection out^T [C, T]; the host sums the two
head-group partials per batch and adds the bias terms.

v2 (vs v1 baseline 358us):
  * Head-PAIR processing: even local head lives on SBUF partitions 0-63,
    odd head on 64-127. The two K=64 scores matmuls of a pair are emitted
    back-to-back with inferred tile_position (0,0)/(64,0) -> they run
    CONCURRENTLY in the PE array (row-group tiling), and their LDWEIGHTS
    cross-hide against the other row group's matmul.
  * No negI mask matmuls: causal masking is a post-exp DVE multiply of the
    128-wide diagonal block only (PE -21us, DVE +16us).
  * qkv projection (phase B), attention (phase C) and out-projection are
    interleaved in emission order so the Tile scheduler can fill PE stalls
    (C is ACT-exp-bound in bursts) with independent B/proj matmuls.
  * x DMA-transposes split across the two HWDGE queues (sync+scalar) and
    issued first: PE starts ~6us in instead of ~24us.
  * normalize reads U'/rowsum directly from PSUM (no staging copies).

fp16 datapath (fp32 PSUM accumulation everywhere, fp32 softmax denominator).

Per-core pipeline per head pair u (heads 2u, 2u+1), per 512-wide i-chunk ic:
  S^T[j, i] = k_h^T q_h  for both heads -> one psum tile [128, 2, 512]
  P = exp(S^T / 8)       (one ACT op per (jt, ic) sub, both heads)
  diagonal j-tile: P *= keep (DVE, precomputed lower-tri mask)
  U'^T [65, i] (+)= [v_h|1]^T P_h^T  over j-tiles (ones col => rowsum row 64)
  yT[u-block, i] = U'[0:64] * bcast(1/rowsum)
out^T = W_p^T yT -> fp32 psum -> DVE copy -> DMA
"""

import numpy as np

import concourse.bass as bass
import concourse.mybir as mybir
import concourse.tile as tile
from concourse import bacc, bass_utils

B, T, C, H = 4, 2048, 1024, 16
HD = C // H          # 64 head dim
N_CORES = 8
HG = H // 2          # 8 heads per core
CL = HG * HD         # 512 local width of q/k/v
TT = T // 128        # 16 t-tiles
CB = C // 128        # 8 c-tiles
DB = CL // 128       # 4 local-hd tiles
NIC = T // 512       # i-chunks (4)
NP = HG // 2         # head pairs per core (4)

f32 = mybir.dt.float32
f16 = mybir.dt.float16

_PROG_CACHE = {}


def _emit(tc, aps):
    nc = tc.nc
    Exp = mybir.ActivationFunctionType.Exp
    Copy = mybir.ActivationFunctionType.Copy

    x_ap = aps["x"]
    wqk_ap = aps["wqk"]
    wv_ap = aps["wv"]
    wp_ap = aps["wp"]
    bqk_ap = aps["bqk"]
    keep2_ap = aps["keep2"]
    outT_ap = aps["outT"]

    from contextlib import ExitStack

    with ExitStack() as outer:
        const = outer.enter_context(tc.tile_pool(name="const", bufs=1))
        p_xT = outer.enter_context(tc.tile_pool(name="xT", bufs=1))
        p_qkT = outer.enter_context(tc.tile_pool(name="qkT", bufs=1))
        p_v = outer.enter_context(tc.tile_pool(name="vv", bufs=1))
        p_yT = outer.enter_context(tc.tile_pool(name="yT", bufs=1))
        p_w = outer.enter_context(tc.tile_pool(name="wsb", bufs=1))
        p_p = outer.enter_context(tc.tile_pool(name="pp", bufs=8))
        p_rb = outer.enter_context(tc.tile_pool(name="rb", bufs=3))
        p_ost = outer.enter_context(tc.tile_pool(name="ost", bufs=4))
        ps_b = outer.enter_context(tc.tile_pool(name="ps_b", bufs=2, space="PSUM"))
        ps_sc = outer.enter_context(tc.tile_pool(name="ps_sc", bufs=2, space="PSUM"))
        ps_u = outer.enter_context(tc.tile_pool(name="ps_u", bufs=2, space="PSUM"))

        # ---- DMAs: x transposes are the PE critical path -> they own both
        # HWDGE queues (sync+scalar), with t-chunk 0 split across the two so
        # its descriptor GENERATION (~6us each) runs in parallel. All weight
        # loads go through the gpsimd SWDGE queue (cheap generation).
        # x transposes: XBAR transfer is fast (~14ns/16x128 tile); the cost is
        # per-instruction DGE latency, which pipelines across the two HWDGE
        # queues. Small first pieces unblock B(0) early.
        # Warm-up tile memset leads the otherwise-idle gpsimd queue.
        warm = const.tile([128, 512], f16)
        nc.gpsimd.memset(warm[:], 0.0)
        # B(0) needs exactly chunk0-transposed + the wqk q-half (2MB): those
        # two lead their queues; ALL other input DMAs queue strictly behind
        # them so they don't steal DMA-fabric bandwidth from the critical
        # pair. XBAR transposes serialize FIFO across queues -> all on scalar.
        xT = p_xT.tile([128, CB, T], f16)
        nc.scalar.dma_start_transpose(xT[:, :, 0:256], x_ap[0:256, :])
        nc.scalar.dma_start_transpose(xT[:, :, 256:512], x_ap[256:512, :])
        nc.scalar.dma_start_transpose(xT[:, :, 512:1024], x_ap[512:1024, :])
        nc.scalar.dma_start_transpose(xT[:, :, 1024:1536], x_ap[1024:1536, :])
        nc.scalar.dma_start_transpose(xT[:, :, 1536:2048], x_ap[1536:2048, :])
        wqk_sb = p_w.tile([128, CB, CB * 128], f16)  # [c-part, cb, co*128+q]
        nc.sync.dma_start(
            wqk_sb[:, :, 0 : 4 * 128],
            wqk_ap[:, 0 : 4 * 128].rearrange("(cb p) n -> p cb n", p=128),
        )
        bqk = const.tile([128, CB], f32)
        nc.sync.dma_start(bqk[:], bqk_ap.rearrange("co p -> p co"))
        # k-half on the gpsimd queue: runs parallel to sync's q-half, so the
        # first scores (which need k) aren't gated behind wv/wp.
        nc.gpsimd.dma_start(
            wqk_sb[:, :, 4 * 128 : 8 * 128],
            wqk_ap[:, 4 * 128 : 8 * 128].rearrange("(cb p) n -> p cb n", p=128),
        )
        keep2 = const.tile([128, 2, 128], f16)   # keep[j, ix, i] = (j <= i)
        nc.gpsimd.dma_start(keep2[:], keep2_ap)
        wv_sb = p_w.tile([128, CB, CL], f16)
        nc.sync.dma_start(wv_sb[:], wv_ap.rearrange("(cb p) n -> p cb n", p=128))
        wp_sb = p_w.tile([128, DB, C], f16)
        nc.sync.dma_start(wp_sb[:], wp_ap.rearrange("(db p) c -> p db c", p=128))

        # ---- PE warm-up: dep-free matmuls on the zeroed tile keep the PE
        # busy from t~6us so HAM un-throttles to 2.4 GHz before real work
        # arrives, and the array never sits cold waiting for the first chunk.
        wups = ps_u.tile([HD + 1, 512], f32, tag="u", name="wups")
        for _ in range(26):
            nc.tensor.matmul(
                wups[:], warm[:, 0 : HD + 1], warm[:], start=True, stop=True
            )

        # per-(co, tn) qkT tiles, per-jt v' tiles, per-tn yT tiles
        qkT = {}
        for co in range(CB):
            for tn in range(NIC):
                qkT[(co, tn)] = p_qkT.tile(
                    [128, 512], f16, tag=f"qkT_{co}_{tn}", name=f"qkT_{co}_{tn}"
                )
        vv = {}
        for jt in range(TT):
            vv[jt] = p_v.tile([128, HG, HD + 1], f16, tag=f"vv_{jt}", name=f"vv_{jt}")
            nc.vector.memset(vv[jt][:, :, HD : HD + 1], 1.0)
        yTn = {}
        for tn in range(NIC):
            yTn[tn] = p_yT.tile([128, DB, 512], f16, tag=f"yT_{tn}", name=f"yT_{tn}")

        def emit_qkT_group(co, tn):
            ps = ps_b.tile([128, 512], f32, tag="bps")
            for cb in range(CB):
                nc.tensor.matmul(
                    ps[:],
                    wqk_sb[:, cb, co * 128 : (co + 1) * 128],
                    xT[:, cb, tn * 512 : (tn + 1) * 512],
                    start=(cb == 0),
                    stop=(cb == CB - 1),
                )
            nc.vector.tensor_scalar_add(qkT[(co, tn)][:], ps[:], bqk[:, co : co + 1])

        def emit_vv_group(tt):
            ps = ps_b.tile([128, CL], f32, tag="bps")
            for cb in range(CB):
                nc.tensor.matmul(
                    ps[:],
                    xT[:, cb, tt * 128 : (tt + 1) * 128],
                    wv_sb[:, cb, :],
                    start=(cb == 0),
                    stop=(cb == CB - 1),
                )
            nc.vector.tensor_copy(
                vv[tt][:, :, 0:HD], ps.rearrange("p (h d) -> p h d", d=HD)
            )

        def normalize(ups, ic, u, r0, on_act=False):
            rs = p_rb.tile([1, 512], f32, tag="rs", name="rs")
            if on_act:  # kernel tail: ACT is idle, DVE is the chokepoint
                nc.scalar.activation(rs[:], ups[HD : HD + 1, :], Copy)
            else:
                nc.vector.tensor_copy(rs[:], ups[HD : HD + 1, :])
            rr = p_rb.tile([1, 512], f32, tag="rr", name="rr")
            nc.vector.reciprocal_approx_fast(rr[:], rs[:])
            rb = p_rb.tile([HD, 512], f32, tag="rb", name="rb")
            nc.gpsimd.partition_broadcast(rb[:], rr[0:1, :], channels=HD)
            nc.vector.tensor_mul(yTn[ic][r0 : r0 + HD, u, :], ups[0:HD, :], rb[:])

        def emit_pair(u, ic):
            """Attention for head pair (2u, 2u+1) over i-chunk ic. The odd
            head's P@V lags LAG j-tiles so the even head's accumulator closes
            early and its normalize chain overlaps remaining PE work."""
            co_q = u
            co_k = 4 + u
            nj = 4 * ic + 4
            lag = min(5, nj - 1)
            ups_e = ps_u.tile([HD + 1, 512], f32, tag="u", name=f"ue_{u}_{ic}")
            ups_o = ps_u.tile([HD + 1, 512], f32, tag="u", name=f"uo_{u}_{ic}")
            pts = {}

            def av(ix, ups, jt):
                m = jt % 4
                lo = 128 * m if ic == jt // 4 else 0
                nc.tensor.matmul(
                    ups[:, lo:512],
                    vv[jt][:, 2 * u + ix, :],
                    pts[jt][:, ix, lo:512],
                    start=(jt == 0),
                    stop=(jt == nj - 1),
                )

            for jt in range(nj):
                m = jt % 4
                diag = ic == jt // 4
                lo = 128 * m if diag else 0
                psg = ps_sc.tile([128, 2, 512], f32, tag="sc")
                for ix in range(2):
                    r0 = 64 * ix
                    nc.tensor.matmul(
                        psg[:, ix, lo:512],
                        qkT[(co_k, jt // 4)][r0 : r0 + 64, m * 128 : (m + 1) * 128],
                        qkT[(co_q, ic)][r0 : r0 + 64, lo:512],
                        start=True,
                        stop=True,
                    )
                pt = p_p.tile([128, 2, 512], f16, tag="p")
                pts[jt] = pt
                nc.scalar.activation(
                    pt[:, 0:2, lo:512], psg[:, 0:2, lo:512], Exp, scale=1.0 / np.sqrt(HD)
                )
                if diag:  # zero strictly-upper part of the diagonal block
                    nc.vector.tensor_mul(
                        pt[:, 0:2, lo : lo + 128],
                        pt[:, 0:2, lo : lo + 128],
                        keep2[:, 0:2, :],
                    )
                av(0, ups_e, jt)
                if jt >= lag:
                    av(1, ups_o, jt - lag)
            tail = ic == NIC - 1 and u == NP - 1
            normalize(ups_e, ic, u, 0, on_act=tail)
            for jt in range(nj - lag, nj):
                av(1, ups_o, jt)
            normalize(ups_o, ic, u, 64, on_act=tail)

        def emit_proj(tn, cos, on_act=False, split_last=False):
            for co in cos:
                psp = ps_b.tile([128, 512], f32, tag="bps")
                cs = slice(co * 128, (co + 1) * 128)
                if split_last:
                    # db=3 (the last-finishing pair) contracted in two K=64
                    # halves bracketing the full-row MMs: the top half only
                    # needs the even head's normalize, and no two row-disjoint
                    # MMs are adjacent within the group (PSUM-bank rule).
                    nc.tensor.matmul(
                        psp[:], wp_sb[0:64, 3, cs], yTn[tn][0:64, 3, :],
                        start=True, stop=False,
                    )
                    for db in range(DB - 1):
                        nc.tensor.matmul(
                            psp[:], wp_sb[:, db, cs], yTn[tn][:, db, :],
                            start=False, stop=False,
                        )
                    nc.tensor.matmul(
                        psp[:], wp_sb[64:128, 3, cs], yTn[tn][64:128, 3, :],
                        start=False, stop=True,
                    )
                else:
                    for db in range(DB):
                        nc.tensor.matmul(
                            psp[:],
                            wp_sb[:, db, cs],
                            yTn[tn][:, db, :],
                            start=(db == 0),
                            stop=(db == DB - 1),
                        )
                ot = p_ost.tile([128, 512], f32, tag="ot")
                if on_act:  # tail groups: exp is done, ACT is idle, DVE is not
                    nc.scalar.activation(ot[:], psp[:], Copy)
                else:
                    nc.vector.tensor_copy(ot[:], psp[:])
                nc.sync.dma_start(
                    outT_ap[co * 128 : (co + 1) * 128, tn * 512 : (tn + 1) * 512],
                    ot[:],
                )

        # ---- B(0) up front; then C(ic) with B(ic+1)/proj(ic-1) interleaved
        for co in range(CB):
            emit_qkT_group(co, 0)
        for tt in range(4):
            emit_vv_group(tt)

        for ic in range(NIC):
            for u in range(NP):
                emit_pair(u, ic)
                if ic < NIC - 1:
                    emit_qkT_group(2 * u, ic + 1)
                    emit_qkT_group(2 * u + 1, ic + 1)
                    emit_vv_group(4 * (ic + 1) + u)
                if ic >= 1:
                    emit_proj(ic - 1, [u])
            # second half of the previous chunk's projection lands after the
            # last pair: dep-free PE filler over the final normalize chain
            if ic >= 1:
                emit_proj(ic - 1, [4, 5, 6, 7], on_act=(ic == NIC - 1))
        emit_proj(NIC - 1, range(CB), on_act=True, split_last=True)


def _build_program():
    nc = bacc.Bacc("TRN2", target_bir_lowering=False, debug=False, num_devices=N_CORES)
    aps = {
        "x": nc.dram_tensor("x", [T, C], f16, kind="ExternalInput").ap(),
        "wqk": nc.dram_tensor("wqk", [C, CB * 128], f16, kind="ExternalInput").ap(),
        "wv": nc.dram_tensor("wv", [C, CL], f16, kind="ExternalInput").ap(),
        "wp": nc.dram_tensor("wp", [CL, C], f16, kind="ExternalInput").ap(),
        "bqk": nc.dram_tensor("bqk", [CB, 128], f32, kind="ExternalInput").ap(),
        "keep2": nc.dram_tensor("keep2", [128, 2, 128], f16, kind="ExternalInput").ap(),
        "outT": nc.dram_tensor("outT", [C, T], f32, kind="ExternalOutput").ap(),
    }
    with tile.TileContext(nc) as tc:
        _emit(tc, aps)
    nc.compile()
    return nc


def get_program():
    if "nc" not in _PROG_CACHE:
        _PROG_CACHE["nc"] = _build_program()
    return _PROG_CACHE["nc"]


def _host_consts():
    j = np.arange(128)[:, None]
    i = np.arange(128)[None, :]
    keep = (j <= i).astype(np.float16)          # 1 => keep
    keep2 = np.stack([keep, keep], axis=1)      # [128, 2, 128]
    return np.ascontiguousarray(keep2)


def make_in_maps(x, W_attn, b_attn, W_proj):
    """Build the 8 per-core input maps. Core index = 2*batch + head_group."""
    keep2 = _host_consts()
    in_maps = []
    for core in range(N_CORES):
        b = core // 2
        g = core % 2
        wq = W_attn[:, g * CL : (g + 1) * CL]
        wk = W_attn[:, C + g * CL : C + (g + 1) * CL]
        wqk = np.concatenate([wq, wk], axis=1)  # [C, 1024], cols = co*128+q
        wv = W_attn[:, 2 * C + g * CL : 2 * C + (g + 1) * CL]
        bqk = np.concatenate(
            [b_attn[g * CL : (g + 1) * CL], b_attn[C + g * CL : C + (g + 1) * CL]]
        ).reshape(CB, 128)
        in_maps.append(
            {
                "x": np.ascontiguousarray(x[b]).astype(np.float16),
                "wqk": np.ascontiguousarray(wqk).astype(np.float16),
                "wv": np.ascontiguousarray(wv).astype(np.float16),
                "wp": np.ascontiguousarray(W_proj[g * CL : (g + 1) * CL, :]).astype(
                    np.float16
                ),
                "bqk": np.ascontiguousarray(bqk).astype(np.float32),
                "keep2": keep2,
            }
        )
    return in_maps


def run(x, W_attn, b_attn, W_proj, b_proj, trace=False):
    nc = get_program()
    in_maps = make_in_maps(x, W_attn, b_attn, W_proj)
    res = bass_utils.run_bass_kernel_spmd(
        nc, in_maps, core_ids=list(range(N_CORES)), trace=trace
    )
    # combine: out[b] = sum_g outT_{2b+g}^T + (bv_g @ Wp_g summed) + b_proj
    corr = b_proj.astype(np.float64).copy()
    for g in range(2):
        bv_g = b_attn[2 * C + g * CL : 2 * C + (g + 1) * CL]
        corr += bv_g.astype(np.float64) @ W_proj[g * CL : (g + 1) * CL, :].astype(
            np.float64
        )
    out = np.empty((B, T, C), np.float32)
    for b in range(B):
        acc = (
            res.results[2 * b]["outT"].T.astype(np.float64)
            + res.results[2 * b + 1]["outT"].T.astype(np.float64)
            + corr
        )
        out[b] = acc.astype(np.float32)
    return out, res


def kernel(x, W_attn, b_attn, W_proj, b_proj):
    x = np.asarray(x, np.float32)
    W_attn = np.asarray(W_attn, np.float32)
    b_attn = np.asarray(b_attn, np.float32)
    W_proj = np.asarray(W_proj, np.float32)
    b_proj = np.asarray(b_proj, np.float32)
    out, _ = run(x, W_attn, b_attn, W_proj, b_proj)
    return out
